# revision 10
# baseline (speedup 1.0000x reference)
"""Trainium2 Bass kernel for the CGA sandwich pipeline (nn_CGAPipeline).

out = decode( (V * encode(x)) * ~V ) over N=2^21 points, data-parallel over
8 NeuronCores.

Algorithm notes (see blade-index math below):
- Even blades of Cl(4,1) under XOR of codes form a group; rank(code) =
  code >> 1 maps the reference's sorted blade order onto a 4-bit XOR index
  space. All Cayley contractions become signed XOR-permutations.
- Layout on chip: feature-on-partition "FOP" packing, 8 point-groups x 16
  blades = 128 partitions, points along the free dim (C=512 per compute
  tile). PE does transposes + signed-permutation matmuls + broadcast
  matmuls (float32r, 1 cyc/row at free>=256); DVE/GPSIMD do the per-point
  products; ACT does PSUM->SBUF evacuations; decode happens back in
  point-on-partition layout.
"""
import sys

sys.path.insert(0, "/opt/trn_rl_repo")

import numpy as np

import concourse.bacc as bacc
import concourse.bass as bass
import concourse.mybir as mybir
import concourse.tile as tile
from concourse.bass_types import AP
from concourse.bass_utils import run_bass_kernel_spmd

F32 = mybir.dt.float32
F32R = mybir.dt.float32r
ALU = mybir.AluOpType

# ----------------------------------------------------------------------------
# Cl(4,1) tables (rank-indexed; rank == position in the sorted blade order)
# ----------------------------------------------------------------------------
_METRIC = [1.0, 1.0, 1.0, 1.0, -1.0]


def _popcount(x):
    return bin(x).count("1")


def _blade_mul(a, b):
    s = 0
    t = a >> 1
    while t:
        s += _popcount(t & b)
        t >>= 1
    sign = -1.0 if (s & 1) else 1.0
    for i in range(5):
        if (a >> i) & 1 and (b >> i) & 1:
            sign *= _METRIC[i]
    return a ^ b, sign


def _rev_sign(b):
    g = _popcount(b)
    return -1.0 if (g * (g - 1) // 2) % 2 else 1.0


def _E_code(i):
    return (i << 1) | (_popcount(i) & 1)


def _O_code(j):
    return (j << 1) | ((_popcount(j) + 1) & 1)


_KAPPAS = [1, 2, 4, 8, 16]  # e1..e5 blade codes; channel c = kappa >> 1

_s1 = np.zeros((16, 5), np.float32)
for _pi, _kp in enumerate(_KAPPAS):
    _c = _kp >> 1
    for _j in range(16):
        _code, _sg = _blade_mul(_E_code(_j ^ _c), _kp)
        assert _code == _O_code(_j)
        _s1[_j, _pi] = _sg

_s2 = np.zeros((16, 5), np.float32)
for _qi, _kq in enumerate(_KAPPAS):
    _c = _kq >> 1
    for _j in range(16):
        _ip = _j ^ _c
        _code, _sg = _blade_mul(_O_code(_j), _E_code(_ip))
        assert _code == _kq
        _s2[_j, _qi] = _sg * _rev_sign(_E_code(_ip))


# ----------------------------------------------------------------------------
# Weight matrices (all lhsT layout: out = lhsT.T @ rhs)
# ----------------------------------------------------------------------------
def _blockdiag8(m16):
    w = np.zeros((128, 128), np.float32)
    for g in range(8):
        w[16 * g : 16 * g + 16, 16 * g : 16 * g + 16] = m16
    return w


def _perm_m16(coef):  # coef(j, i) -> m16[i, j]
    m = np.zeros((16, 16), np.float32)
    for j in range(16):
        for i in range(16):
            m[i, j] = coef(j, i)
    return m


# X_c(V)[j] = v[j^c]
W_X1 = _blockdiag8(_perm_m16(lambda j, i: 1.0 if i == (j ^ 1) else 0.0))
W_X2 = _blockdiag8(_perm_m16(lambda j, i: 1.0 if i == (j ^ 2) else 0.0))
# Vinf[j] = s1(j,e4) v[j^4] + s1(j,e5) v[j^8]
W_VINF = _blockdiag8(
    _perm_m16(
        lambda j, i: (_s1[j, 3] if i == (j ^ 4) else 0.0)
        + (_s1[j, 4] if i == (j ^ 8) else 0.0)
    )
)
# Cp[j] = -0.5 s1(j,e4) v[j^4] + 0.5 s1(j,e5) v[j^8]
W_CP = _blockdiag8(
    _perm_m16(
        lambda j, i: (-0.5 * _s1[j, 3] if i == (j ^ 4) else 0.0)
        + (0.5 * _s1[j, 4] if i == (j ^ 8) else 0.0)
    )
)
# D[j] = s2(j,e5) v[j^8] - s2(j,e4) v[j^4]
W_D = _blockdiag8(
    _perm_m16(
        lambda j, i: (_s2[j, 4] if i == (j ^ 8) else 0.0)
        + (-_s2[j, 3] if i == (j ^ 4) else 0.0)
    )
)

# bcast weights: B_p[16g+j] = sgn(j) * xt_channel_p ; xt rows are (4g+ch)
# channels: 0->x1, 1->x2, 2->x3, 3->sum(x^2) (h channel carries 0.5 factor)
W_B = []
for _p in range(4):
    w = np.zeros((32, 128), np.float32)
    for g in range(8):
        for j in range(16):
            if _p < 3:
                w[4 * g + _p, 16 * g + j] = _s1[j, _p]
            else:
                w[4 * g + 3, 16 * g + j] = 0.5  # h = 0.5*sum(x^2); signs in W_VINF
    W_B.append(w)

# Wsum_r: out_fop row (8r + g) = sum_j s2(j,r) * Z_r[16g+j]   (r<3)
#         row (24 + g)        = sum_j          Zs[16g+j]
W_SUM = []
for _r in range(4):
    w = np.zeros((128, 32), np.float32)
    for g in range(8):
        for j in range(16):
            w[16 * g + j, 8 * _r + g] = _s2[j, _r] if _r < 3 else 1.0
    W_SUM.append(w)

IDENT128 = np.eye(128, dtype=np.float32)
IDENT32 = np.eye(32, dtype=np.float32)


# Single packed weight blob: one DMA -> one semaphore for all matmul weights.
_WOFF = {}
_wcols = 0
def _wadd(name, arr, rows):
    global _wcols
    _WOFF[name] = (_wcols, arr.shape[1], rows)
    _wcols += arr.shape[1]
for _n, _a in [("ident128", IDENT128), ("w_x1", W_X1), ("w_x2", W_X2),
               ("w_vinf", W_VINF), ("w_cp", W_CP), ("w_d", W_D),
               ("w_s0", W_SUM[0]), ("w_s1", W_SUM[1]), ("w_s2", W_SUM[2]),
               ("w_s3", W_SUM[3])]:
    _wadd(_n, _a, 128)
for _n, _a in [("w_b0", W_B[0]), ("w_b1", W_B[1]), ("w_b2", W_B[2]),
               ("w_b3", W_B[3]), ("ident32", IDENT32)]:
    _wadd(_n, _a, 32)
WPACK = np.zeros((128, _wcols), np.float32)
for _n, _a in [("ident128", IDENT128), ("w_x1", W_X1), ("w_x2", W_X2),
               ("w_vinf", W_VINF), ("w_cp", W_CP), ("w_d", W_D),
               ("w_s0", W_SUM[0]), ("w_s1", W_SUM[1]), ("w_s2", W_SUM[2]),
               ("w_s3", W_SUM[3]), ("w_b0", W_B[0]), ("w_b1", W_B[1]),
               ("w_b2", W_B[2]), ("w_b3", W_B[3]), ("ident32", IDENT32)]:
    _o, _w, _r = _WOFF[_n]
    WPACK[:_r, _o:_o + _w] = _a

WEIGHTS = {"wpack": WPACK}


# ----------------------------------------------------------------------------
# Geometry
# ----------------------------------------------------------------------------
N_TOTAL = 2097152
N_CORES = 8
NPC = N_TOTAL // N_CORES          # 262144 points per core
PPM = 16384                       # points per macro tile (128 part x 128 pts)
NMACRO = NPC // PPM               # 16
NT = 16                           # transpose chunks (1024 pts) per macro
CT = 4                            # compute tiles per macro
C = 512                           # free columns per compute tile (4096 pts)


def _cap(t_ap, off, dims):
    """Custom free-dim AP on a tile: keep partition dim, replace free dims."""
    p = t_ap.ap[0]
    return AP(t_ap.tensor, t_ap.offset + off, [list(p)] + [list(d) for d in dims])


def build_bass():
    nc = bacc.Bacc("TRN2")

    v_d = nc.dram_tensor("versor", [NMACRO, 128, 2048], F32R, kind="ExternalInput")
    x_d = nc.dram_tensor("x", [NMACRO, 128, 384], F32, kind="ExternalInput")
    o_d = nc.dram_tensor("out", [NMACRO, 128, 512], F32, kind="ExternalOutput")
    w_d = {
        name: nc.dram_tensor(name, list(arr.shape), F32R, kind="ExternalInput")
        for name, arr in WEIGHTS.items()
    }

    from contextlib import ExitStack

    with tile.TileContext(nc) as tc, ExitStack() as ctx:
        wpool = ctx.enter_context(tc.tile_pool(name="wpool", bufs=1))
        wpack_sb = wpool.tile(list(WPACK.shape), F32R, tag="wpack")
        nc.sync.dma_start(wpack_sb[:], w_d["wpack"][:])

        def wap(name):  # weights live as f32r slices of the packed blob
            off, width, rows = _WOFF[name]
            return wpack_sb[:rows, off : off + width]

        i128 = wap("ident128")
        i32 = wap("ident32")

        io_pool = ctx.enter_context(tc.tile_pool(name="io", bufs=2))
        pre_pool = ctx.enter_context(tc.tile_pool(name="pre", bufs=2))
        sb_pool = ctx.enter_context(tc.tile_pool(name="work", bufs=2))
        # PSUM: 8 banks total; every tile rounds up to one bank.
        ps_ev = ctx.enter_context(tc.tile_pool(name="ps_ev", bufs=2, space="PSUM"))
        ps_b = ctx.enter_context(tc.tile_pool(name="ps_b", bufs=2, space="PSUM"))
        ps_cpd = ctx.enter_context(tc.tile_pool(name="ps_cpd", bufs=2, space="PSUM"))
        ps_of = ctx.enter_context(tc.tile_pool(name="ps_of", bufs=1, space="PSUM"))
        ps_out = ctx.enter_context(tc.tile_pool(name="ps_out", bufs=1, space="PSUM"))

        for m in range(NMACRO):
            v_sb = io_pool.tile([128, 2048], F32R, tag="v_sb")
            nc.sync.dma_start(v_sb[:], v_d[m])
            x_sb = io_pool.tile([128, 384], F32, tag="x_sb")
            nc.sync.dma_start(x_sb[:], x_d[m])

            # ---- POP-side precompute: h' = sum(x^2), xt_pop (t,b,ch) ----
            sq = pre_pool.tile([128, 384], F32, tag="sq")
            nc.vector.tensor_mul(sq[:], x_sb[:], x_sb[:])
            hh = pre_pool.tile([128, 128], F32, tag="hh")
            nc.vector.tensor_add(hh[:], sq[:, 0:384:3], sq[:, 1:384:3])
            nc.vector.tensor_add(hh[:], hh[:], sq[:, 2:384:3])

            xt_pop = pre_pool.tile([128, 512], F32R, tag="xt_pop")
            # copy x channels into (t,b,ch<3) slots
            nc.vector.tensor_copy(
                _cap(xt_pop[:], 0, [[32, 16], [4, 8], [1, 3]]),
                _cap(x_sb[:], 0, [[24, 16], [3, 8], [1, 3]]),
            )
            # h channel into ch=3 slots
            nc.vector.tensor_copy(
                _cap(xt_pop[:], 3, [[32, 16], [4, 8]]),
                _cap(hh[:], 0, [[8, 16], [1, 8]]),
            )

            # output staging (per macro)
            out_pop = ps_out.tile([128, 512], F32, tag="out_pop")

            for ict in range(CT):
                # ---- transposes ----
                # PSUM budget is 8 banks; "ev" tiles are evacuated promptly and
                # rotate through 2 banks.
                x0_ps = ps_ev.tile([128, C], F32, tag="ev")
                xt_ps = ps_ev.tile([32, C], F32, tag="ev")
                for tt in range(4):
                    t = ict * 4 + tt
                    nc.tensor.transpose(
                        x0_ps[:, tt * 128 : tt * 128 + 128].bitcast(F32R),
                        v_sb[:, t * 128 : t * 128 + 128],
                        i128,
                    )
                    nc.tensor.transpose(
                        xt_ps[:, tt * 128 : tt * 128 + 128].bitcast(F32R),
                        xt_pop[:, t * 32 : t * 32 + 32],
                        i128,
                    )

                v_fop = sb_pool.tile([128, C], F32R, tag="v_fop")
                nc.scalar.copy(v_fop[:], x0_ps[:])
                xt_fop = sb_pool.tile([32, C], F32R, tag="xt_fop")
                nc.scalar.copy(xt_fop[:], xt_ps[:])

                vr = v_fop[:]
                xtr = xt_fop[:]
                vf = v_fop[:].bitcast(F32)

                # ---- stage-1 structure matmuls ----
                x1_ps = ps_ev.tile([128, C], F32, tag="ev")
                nc.tensor.matmul(x1_ps[:], wap("w_x1"), vr, start=True, stop=True)
                x2_ps = ps_ev.tile([128, C], F32, tag="ev")
                nc.tensor.matmul(x2_ps[:], wap("w_x2"), vr, start=True, stop=True)
                vinf_ps = ps_ev.tile([128, C], F32, tag="ev")
                nc.tensor.matmul(vinf_ps[:], wap("w_vinf"), vr, start=True, stop=True)
                cp_ps = ps_cpd.tile([128, C], F32, tag="cpd")
                nc.tensor.matmul(cp_ps[:], wap("w_cp"), vr, start=True, stop=True)
                d_ps = ps_cpd.tile([128, C], F32, tag="cpd")
                nc.tensor.matmul(d_ps[:], wap("w_d"), vr, start=True, stop=True)

                x1_sb = sb_pool.tile([128, C], F32, tag="x1_sb")
                nc.scalar.copy(x1_sb[:], x1_ps[:])
                x2_sb = sb_pool.tile([128, C], F32, tag="x2_sb")
                nc.scalar.copy(x2_sb[:], x2_ps[:])
                vinf_sb = sb_pool.tile([128, C], F32, tag="vinf_sb")
                nc.scalar.copy(vinf_sb[:], vinf_ps[:])

                # ---- bcast matmuls ----
                b_ps = []
                for p in range(4):
                    bp = ps_b.tile([128, C], F32, tag="b")
                    nc.tensor.matmul(bp[:], wap(f"w_b{p}"), xtr, start=True, stop=True)
                    b_ps.append(bp)

                # ---- stage-1 products + accumulation ----
                t0 = sb_pool.tile([128, C], F32, tag="t0")
                nc.vector.tensor_mul(t0[:], vf, b_ps[0][:])
                t1 = sb_pool.tile([128, C], F32, tag="t1")
                nc.vector.tensor_mul(t1[:], x1_sb[:], b_ps[1][:])
                t2 = sb_pool.tile([128, C], F32, tag="t2")
                nc.vector.tensor_mul(t2[:], x2_sb[:], b_ps[2][:])
                t3 = sb_pool.tile([128, C], F32, tag="t3")
                nc.vector.tensor_mul(t3[:], vinf_sb[:], b_ps[3][:])

                a1 = sb_pool.tile([128, C], F32, tag="a1")
                nc.gpsimd.tensor_add(a1[:], t0[:], t1[:])
                a2 = sb_pool.tile([128, C], F32, tag="a2")
                nc.gpsimd.tensor_add(a2[:], t2[:], t3[:])
                a3 = sb_pool.tile([128, C], F32, tag="a3")
                nc.gpsimd.tensor_add(a3[:], a1[:], a2[:])
                mx = sb_pool.tile([128, C], F32, tag="mx")
                nc.vector.tensor_add(mx[:], a3[:], cp_ps[:])

                # ---- stage-2 products ----
                z0 = sb_pool.tile([128, C], F32R, tag="z0")
                nc.vector.tensor_mul(z0[:], mx[:], vf)
                z1 = sb_pool.tile([128, C], F32R, tag="z1")
                nc.vector.tensor_mul(z1[:], mx[:], x1_sb[:])
                z2 = sb_pool.tile([128, C], F32R, tag="z2")
                nc.vector.tensor_mul(z2[:], mx[:], x2_sb[:])
                zs = sb_pool.tile([128, C], F32R, tag="zs")
                nc.vector.tensor_mul(zs[:], mx[:], d_ps[:])

                # ---- output contraction ----
                out_fop = ps_of.tile([32, C], F32, tag="out_fop")
                for r, z in enumerate([z0, z1, z2, zs]):
                    nc.tensor.matmul(
                        out_fop[:],
                        wap(f"w_s{r}"),
                        z[:],
                        start=(r == 0),
                        stop=(r == 3),
                    )

                out_evac = sb_pool.tile([32, C], F32R, tag="out_evac")
                nc.scalar.copy(out_evac[:], out_fop[:])

                for tt in range(4):
                    t = ict * 4 + tt
                    nc.tensor.transpose(
                        out_pop[:, t * 32 : t * 32 + 32].bitcast(F32R),
                        out_evac[:, tt * 128 : tt * 128 + 128],
                        i32,
                    )

            # ---- decode (POP layout: free = (t, 8r+b), s at 24..31) ----
            rec = pre_pool.tile([128, 128], F32, tag="rec")
            nc.vector.reciprocal(
                rec[:], _cap(out_pop[:], 24, [[32, 16], [1, 8]])
            )
            out_sb = pre_pool.tile([128, 512], F32, tag="out_sb")
            nc.vector.tensor_mul(
                _cap(out_sb[:], 0, [[32, 16], [4, 8], [1, 3]]),
                _cap(out_pop[:], 0, [[32, 16], [1, 8], [8, 3]]),
                _cap(rec[:], 0, [[8, 16], [1, 8], [0, 3]]),
            )
            # raw s travels with the output for host-side conditioning fixup
            nc.vector.tensor_copy(
                _cap(out_sb[:], 3, [[32, 16], [4, 8]]),
                _cap(out_pop[:], 24, [[32, 16], [1, 8]]),
            )
            nc.sync.dma_start(o_d[m], out_sb[:])

    nc.compile()
    return nc


_NC_CACHE = None


def _get_nc():
    global _NC_CACHE
    if _NC_CACHE is None:
        _NC_CACHE = build_bass()
    return _NC_CACHE


def kernel(versor: np.ndarray, x: np.ndarray) -> np.ndarray:
    versor = np.ascontiguousarray(versor, dtype=np.float32)
    x = np.ascontiguousarray(x, dtype=np.float32)
    nc = _get_nc()

    in_maps = []
    for c in range(N_CORES):
        sl = slice(c * NPC, (c + 1) * NPC)
        im = {
            "versor": versor[sl].reshape(NMACRO, 128, 2048),
            "x": x[sl].reshape(NMACRO, 128, 384),
        }
        for name, arr in WEIGHTS.items():
            im[name] = arr
        in_maps.append(im)

    res = run_bass_kernel_spmd(nc, in_maps, core_ids=list(range(N_CORES)))
    out4 = np.concatenate(
        [res.results[c]["out"].reshape(NPC, 4) for c in range(N_CORES)], axis=0
    )
    out = np.ascontiguousarray(out4[:, :3])
    sk = out4[:, 3]

    # Conditioning fixup: the on-chip pipeline contracts products of
    # magnitude ~h*|v|^2 at float32r precision (~2^-12); points with tiny
    # denominator s or large h amplify that rounding beyond fp32 level.
    # Recompute those few points exactly.
    h = 0.5 * np.einsum("ij,ij->i", x, x)
    flag = (np.abs(sk) < 0.5) | (h > 6.0)
    if np.any(flag):
        out[flag] = _exact_ref(versor[flag], x[flag])
    return out.astype(np.float32)


def _exact_ref(versor, x):
    v = versor.astype(np.float64)
    xf = x.astype(np.float64)
    h = 0.5 * np.sum(xf * xf, axis=1)

    def X(c):
        return v[:, np.arange(16) ^ c]

    T0 = X(0) * (_s1[None, :, 0] * xf[:, 0:1])
    T1 = X(1) * (_s1[None, :, 1] * xf[:, 1:2])
    T2 = X(2) * (_s1[None, :, 2] * xf[:, 2:3])
    Vinf = _s1[None, :, 3] * X(4) + _s1[None, :, 4] * X(8)
    Cp = -0.5 * _s1[None, :, 3] * X(4) + 0.5 * _s1[None, :, 4] * X(8)
    mx = T0 + T1 + T2 + Vinf * h[:, None] + Cp
    D = _s2[None, :, 4] * X(8) - _s2[None, :, 3] * X(4)
    s = np.sum(mx * D, axis=1)
    num = np.stack(
        [np.sum(_s2[None, :, r] * (mx * X(r)), axis=1) for r in range(3)], axis=1
    )
    return (num / s[:, None]).astype(np.float32)


if __name__ == "__main__":
    rng = np.random.default_rng(0)
    v = (0.1 * rng.standard_normal((N_TOTAL, 16))).astype(np.float32)
    v[:, 0] += 1.0
    x = rng.standard_normal((N_TOTAL, 3)).astype(np.float32)
    out = kernel(versor=v, x=x)
    print("kernel ran, out shape", out.shape, out.dtype)
